# revision 5
# baseline (speedup 1.0000x reference)
"""Trainium2 Bass kernel for nn_MultiHeadAttention_89232240541956.

Computes, for B=8, S=4096, H=1024, ATTN=1024, EXT=1152:
    x_ext = [h | broadcast(g) | l]                       [B, S, 1152]
    q = relu(x_ext @ Wq + bq); k = relu(x_ext @ Wk + bk) [B, S, 1024]
    scores = sum(q * k, -1) / 32, masked to -1e9 where mask == 1

Sharding: data-parallel over batch — core b owns batch b.

Key transformations (on top of the 218us baseline, see git-less history in
kernel_baseline_218.py):
  - v (Wv, bv) is dead code in the reference — skipped.
  - g @ Wq[1024:1088] folded into the bias on host; bias folded into the
    matmul as a ones-row against a bias-row.
  - 6 matmul passes per projection per tile (vs 7): fp8 E4M3 DoubleRow
    chunks cover 706 of the 1089 contraction rows (3 passes: 256+256+194),
    bf16 covers the rest (3 passes: 128+128+127 = 318 h + 64 l + bias).
    n8=706 is the minimum fp8 coverage that reaches 6 passes, minimizing
    fp8 quantization noise. Host-simulated (sim_err.py) max rel err
    1.789e-2 (device-validated sim: n8=512 sim 1.570e-2 vs device
    1.579e-2); gate is 2e-2.
  - Partial-height chunks (97/127 partitions) matmul'd directly — no
    zero padding, no wasted DMA bytes (matmul cost depends on N only).
  - Epilogue uses relu(q)*relu(k) == relu(relu(q)*k): ScalarE relu on q
    (PSUM->SBUF bf16), DVE multiply vs raw k PSUM, ScalarE Relu-with-
    accum for the row reduction. One ScalarE pass saved per tile.
  - Head: DMA triggers batched (one per W/x slab) and issued in exact
    first-use order across four engine queues, so the first matmul start
    is trigger+transfer-bound (~1.4us after the entry barrier), not
    serialized-trigger-bound (~5.7us in the baseline).
  - Tail: last tile runs q-pass, then k in halves; q-relu and the h0
    half-epilogue overlap the remaining k matmuls; output DMA'd in two
    slabs (cols 0:28 early).
  - Scales: x*16, W*64 (powers of two: lossless in bf16, in-range for
    TRN E4M3's +-240 max). The 2^10 product scale and 1/32 are divided
    out in the epilogue accum op.
"""

import numpy as np
import ml_dtypes

B, S, H, LOC = 8, 4096, 1024, 64
ATTN = 1024
SBLK = 512                # seq columns per DMA block
NBLK = S // SBLK          # 8
NT = SBLK // 128          # 4 seq tiles (128 tokens) per block
NCOL = S // 128           # 32 output columns

N8 = 706                  # h-dims in fp8 (2x256 full chunks + 194-row chunk)
NC2 = (N8 - 512) // 2     # 97 partitions in the short fp8 chunk
NBH = H - N8              # 318 bf16 h-dims
NJ2 = 62 + LOC + 1        # 127 partitions in the short bf16 chunk (h|l|bias)

BF16 = ml_dtypes.bfloat16
XS = 16.0
WS = 64.0

_CACHE = {}


def _build_nc():
    import concourse.bass as bass
    import concourse.mybir as mybir
    import concourse.tile as tile
    from concourse import bacc

    dt = mybir.dt
    nc = bacc.Bacc(None, target_bir_lowering=False)

    # fp8 x: [blk*128+p, (c*2+j)*512+s] for c<2; short chunk separate.
    xh8_d = nc.dram_tensor("xh8", [NBLK * 128, 2 * 2 * SBLK], dt.float8e4,
                           kind="ExternalInput")
    xh8c2_d = nc.dram_tensor("xh8c2", [NBLK * NC2, 2 * SBLK], dt.float8e4,
                             kind="ExternalInput")
    # bf16 x: [blk*128+p, j*512+s] for j<2; short chunk separate.
    xhb_d = nc.dram_tensor("xhb", [NBLK * 128, 2 * SBLK], dt.bfloat16,
                           kind="ExternalInput")
    xhbj2_d = nc.dram_tensor("xhbj2", [NBLK * NJ2, SBLK], dt.bfloat16,
                             kind="ExternalInput")
    # fp8 W: [p, ((c*2+proj)*2+j)*1024+a] for c<2.
    w8_d = nc.dram_tensor("w8", [128, 2 * 2 * 2 * ATTN], dt.float8e4,
                          kind="ExternalInput")
    w8c2_d = nc.dram_tensor("w8c2", [NC2, 2 * 2 * ATTN], dt.float8e4,
                            kind="ExternalInput")
    # bf16 W: [p, (j*2+proj)*1024+a] for j<2.
    wb_d = nc.dram_tensor("wb", [128, 2 * 2 * ATTN], dt.bfloat16,
                          kind="ExternalInput")
    wbj2_d = nc.dram_tensor("wbj2", [NJ2, 2 * ATTN], dt.bfloat16,
                            kind="ExternalInput")
    out = nc.dram_tensor("out", [128, NCOL], dt.float32, kind="ExternalOutput")

    scale = 1.0 / (32.0 * (XS * WS) ** 2)
    DR = mybir.MatmulPerfMode.DoubleRow
    Relu = mybir.ActivationFunctionType.Relu

    with tile.TileContext(nc) as tc:
        with (
            tc.tile_pool(name="wpool", bufs=1) as wpool,
            tc.tile_pool(name="xpool", bufs=2) as xpool,
            tc.tile_pool(name="epool", bufs=2) as epool,
            tc.tile_pool(name="opool", bufs=1) as opool,
            tc.tile_pool(name="psum", bufs=1, space="PSUM") as psum,
        ):
            w8_sb = wpool.tile([128, 2, 2, 2, ATTN], dt.float8e4, tag="w8")
            w8c2_sb = wpool.tile([NC2, 2, 2, ATTN], dt.float8e4, tag="w8c2")
            wb_sb = wpool.tile([128, 2, 2, ATTN], dt.bfloat16, tag="wb")
            wbj2_sb = wpool.tile([NJ2, 2, ATTN], dt.bfloat16, tag="wbj2")

            xh8_0 = xpool.tile([128, 2, 2, SBLK], dt.float8e4, tag="xh8",
                               name="xh8_0")
            xh8c2_0 = xpool.tile([NC2, 2, SBLK], dt.float8e4, tag="xh8c2",
                                 name="xh8c2_0")
            xhb_0 = xpool.tile([128, 2, SBLK], dt.bfloat16, tag="xhb",
                               name="xhb_0")
            xhbj2_0 = xpool.tile([NJ2, SBLK], dt.bfloat16, tag="xhbj2",
                                 name="xhbj2_0")

            # --- head DMAs ---
            # Concurrent DMAs share bandwidth packet-round-robin, so
            # everything issued at once finishes late together. Instead:
            # three serialized streams in exact use order — W-q on sync,
            # W-k on scalar, x blocks on gpsimd. Within a queue, FIFO
            # order = arrival order, so early chunks land first.
            nc.sync.dma_start(w8_sb[:, 0, 0], w8_d[:, 0:2048])
            nc.gpsimd.dma_start(xh8_0[:], xh8_d[0:128, :])
            nc.scalar.dma_start(w8_sb[:, 0, 1], w8_d[:, 2048:4096])
            nc.sync.dma_start(w8_sb[:, 1, 0], w8_d[:, 4096:6144])
            nc.scalar.dma_start(w8_sb[:, 1, 1], w8_d[:, 6144:8192])
            nc.gpsimd.dma_start(xh8c2_0[:], xh8c2_d[0:NC2, :])
            nc.sync.dma_start(w8c2_sb[:, 0], w8c2_d[:, 0:2048])
            nc.scalar.dma_start(w8c2_sb[:, 1], w8c2_d[:, 2048:4096])
            nc.gpsimd.dma_start(xhb_0[:], xhb_d[0:128, :])
            nc.sync.dma_start(wb_sb[:, 0, 0], wb_d[:, 0:1024])
            nc.scalar.dma_start(wb_sb[:, 0, 1], wb_d[:, 1024:2048])
            nc.sync.dma_start(wb_sb[:, 1, 0], wb_d[:, 2048:3072])
            nc.scalar.dma_start(wb_sb[:, 1, 1], wb_d[:, 3072:4096])
            nc.gpsimd.dma_start(xhbj2_0[:], xhbj2_d[0:NJ2, :])
            nc.sync.dma_start(wbj2_sb[:, 0], wbj2_d[:, 0:1024])
            nc.scalar.dma_start(wbj2_sb[:, 1], wbj2_d[:, 1024:2048])

            score_sb = opool.tile([128, NCOL], dt.float32, tag="score")
            sc2 = opool.tile([128, 2], dt.float32, tag="sc2")

            def mm4(psq, psk, lhs, rhs_q, rhs_k, start, stop, pm):
                """q/k interleaved per half with a shared stationary lhs."""
                for nh in range(2):
                    n0 = nh * 512
                    nc.tensor.matmul(psq[:, n0:n0 + 512], lhs,
                                     rhs_q[:, :, n0:n0 + 512] if pm else
                                     rhs_q[:, n0:n0 + 512],
                                     start=start, stop=stop, perf_mode=pm)
                    nc.tensor.matmul(psk[:, n0:n0 + 512], lhs,
                                     rhs_k[:, :, n0:n0 + 512] if pm else
                                     rhs_k[:, n0:n0 + 512],
                                     start=start, stop=stop, perf_mode=pm)

            def chunk_ops(xh8, xh8c2, xhb, xhbj2, s0):
                """(lhs, rhs_q, rhs_k, perf_mode) per chunk, in order."""
                ops = []
                for c in range(2):
                    ops.append((xh8[:, c, :, s0:s0 + 128],
                                w8_sb[:, c, 0], w8_sb[:, c, 1], DR))
                ops.append((xh8c2[:, :, s0:s0 + 128],
                            w8c2_sb[:, 0], w8c2_sb[:, 1], DR))
                for j in range(2):
                    ops.append((xhb[:, j, s0:s0 + 128],
                                wb_sb[:, j, 0], wb_sb[:, j, 1], None))
                ops.append((xhbj2[:, s0:s0 + 128],
                            wbj2_sb[:, 0], wbj2_sb[:, 1], None))
                return ops

            for blk in range(NBLK):
                if blk == 0:
                    xh8, xh8c2, xhb, xhbj2 = xh8_0, xh8c2_0, xhb_0, xhbj2_0
                else:
                    xh8 = xpool.tile([128, 2, 2, SBLK], dt.float8e4,
                                     tag="xh8", name=f"xh8_{blk}")
                    xh8c2 = xpool.tile([NC2, 2, SBLK], dt.float8e4,
                                       tag="xh8c2", name=f"xh8c2_{blk}")
                    xhb = xpool.tile([128, 2, SBLK], dt.bfloat16,
                                     tag="xhb", name=f"xhb_{blk}")
                    xhbj2 = xpool.tile([NJ2, SBLK], dt.bfloat16,
                                       tag="xhbj2", name=f"xhbj2_{blk}")
                    r0 = blk * 128
                    nc.gpsimd.dma_start(xh8[:], xh8_d[r0:r0 + 128, :])
                    nc.gpsimd.dma_start(xh8c2[:],
                                        xh8c2_d[blk * NC2:(blk + 1) * NC2, :])
                    nc.gpsimd.dma_start(xhb[:], xhb_d[r0:r0 + 128, :])
                    nc.gpsimd.dma_start(xhbj2[:],
                                        xhbj2_d[blk * NJ2:(blk + 1) * NJ2, :])

                for t in range(NT):
                    is_last = blk == NBLK - 1 and t == NT - 1
                    col = blk * NT + t
                    psq = psum.tile([128, ATTN], dt.float32, tag="psq",
                                    bufs=2, name=f"psq_{blk}_{t}")
                    psk = psum.tile([128, ATTN], dt.float32, tag="psk",
                                    bufs=2, name=f"psk_{blk}_{t}")
                    s0 = t * 128
                    ops = chunk_ops(xh8, xh8c2, xhb, xhbj2, s0)

                    if not is_last:
                        for i, (lhs, rq, rk, pm) in enumerate(ops):
                            mm4(psq, psk, lhs, rq, rk,
                                start=(i == 0), stop=(i == 5), pm=pm)
                        qsb = epool.tile([128, ATTN], dt.bfloat16, tag="qsb")
                        nc.scalar.activation(qsb[:], psq[:], Relu)
                        prod = epool.tile([128, ATTN], dt.bfloat16, tag="prod")
                        nc.vector.tensor_mul(prod[:], qsb[:], psk[:])
                        cpy = epool.tile([128, ATTN], dt.bfloat16, tag="cpy")
                        nc.scalar.activation(
                            cpy[:], prod[:], Relu, scale=scale,
                            accum_out=score_sb[:, col:col + 1])
                        if col == NCOL - 5:
                            # early output slab once cols 0..27 are final
                            nc.sync.dma_start(out[:, 0:28], score_sb[:, 0:28])
                    else:
                        # q-pass fully first
                        for i, (lhs, rq, rk, pm) in enumerate(ops):
                            for nh in range(2):
                                n0 = nh * 512
                                nc.tensor.matmul(
                                    psq[:, n0:n0 + 512], lhs,
                                    rq[:, :, n0:n0 + 512] if pm else
                                    rq[:, n0:n0 + 512],
                                    start=(i == 0), stop=(i == 5),
                                    perf_mode=pm)
                        # k-pass per half; q relu + h0 epilogue overlap h1 mms
                        qsb = epool.tile([128, ATTN], dt.bfloat16, tag="qsb")
                        nc.scalar.activation(qsb[:], psq[:], Relu)
                        for nh in range(2):
                            n0 = nh * 512
                            for i, (lhs, rq, rk, pm) in enumerate(ops):
                                nc.tensor.matmul(
                                    psk[:, n0:n0 + 512], lhs,
                                    rk[:, :, n0:n0 + 512] if pm else
                                    rk[:, n0:n0 + 512],
                                    start=(i == 0), stop=(i == 5),
                                    perf_mode=pm)
                            prh = epool.tile([128, 512], dt.bfloat16,
                                             tag="prh", name=f"prh_{nh}")
                            nc.vector.tensor_mul(prh[:], qsb[:, n0:n0 + 512],
                                                 psk[:, n0:n0 + 512])
                            cph = epool.tile([128, 512], dt.bfloat16,
                                             tag="cph", name=f"cph_{nh}")
                            nc.scalar.activation(
                                cph[:], prh[:], Relu, scale=scale,
                                accum_out=sc2[:, nh:nh + 1])
                        nc.vector.tensor_reduce(
                            score_sb[:, col:col + 1], sc2[:],
                            axis=mybir.AxisListType.X, op=mybir.AluOpType.add)
                        nc.sync.dma_start(out[:, 28:32], score_sb[:, 28:32])

    nc.compile()
    return nc


def _get_nc():
    if "nc" not in _CACHE:
        _CACHE["nc"] = _build_nc()
    return _CACHE["nc"]


def prep_in_maps(h, mask, g, l, Wq, bq, Wk, bk, Wv=None, bv=None):
    import concourse.mybir as mybir

    FP8 = mybir.dt.np(mybir.dt.float8e4)

    h = np.asarray(h, dtype=np.float32)
    g = np.asarray(g, dtype=np.float32)
    l_ = np.asarray(l, dtype=np.float32)
    Wq = np.asarray(Wq, dtype=np.float32)
    bq = np.asarray(bq, dtype=np.float32)
    Wk = np.asarray(Wk, dtype=np.float32)
    bk = np.asarray(bk, dtype=np.float32)

    # Fold the per-batch g contribution into the bias (fp32 on host).
    bq_eff = bq[None, :] + g @ Wq[H:H + LOC]            # [B, ATTN]
    bk_eff = bk[None, :] + g @ Wk[H + 0:H + LOC]

    # --- shared weights ---
    w8 = np.empty((128, 2, 2, 2, ATTN), dtype=FP8)
    w8c2 = np.empty((NC2, 2, 2, ATTN), dtype=FP8)
    wb = np.empty((128, 2, 2, ATTN), dtype=BF16)
    wbj2_base = np.empty((NJ2, 2, ATTN), dtype=np.float32)
    for proj, W in ((0, Wq), (1, Wk)):
        W8 = (W[:N8] * WS).astype(FP8)
        # rows c*256 + 2p + j  ->  [c][p][j][a]
        w8[:, :, proj] = W8[:512].reshape(2, 128, 2, ATTN).transpose(1, 0, 2, 3)
        w8c2[:, proj] = W8[512:N8].reshape(NC2, 2, ATTN)
        Wbf = (W[N8:H] * WS).astype(BF16)
        wb[:, :, proj] = Wbf[:256].reshape(2, 128, ATTN).transpose(1, 0, 2)
        wbj2_base[0:62, proj] = W[N8 + 256:H] * WS
        wbj2_base[62:62 + LOC, proj] = W[H + LOC:] * WS
    base = {"w8": w8.reshape(128, -1), "w8c2": w8c2.reshape(NC2, -1),
            "wb": wb.reshape(128, -1)}

    in_maps = []
    for b in range(B):
        m = dict(base)
        hT = h[b].T                                     # [H, S]
        x8 = (hT[:N8] * XS).astype(FP8)                 # [706, S]
        # rows c*256+2p+j, cols blk*512+s -> [blk][p][c][j][s]
        m["xh8"] = np.ascontiguousarray(
            x8[:512].reshape(2, 128, 2, NBLK, SBLK).transpose(3, 1, 0, 2, 4)
        ).reshape(NBLK * 128, -1)
        m["xh8c2"] = np.ascontiguousarray(
            x8[512:N8].reshape(NC2, 2, NBLK, SBLK).transpose(2, 0, 1, 3)
        ).reshape(NBLK * NC2, -1)
        xb = (hT[N8:] * XS).astype(BF16)                # [318, S]
        m["xhb"] = np.ascontiguousarray(
            xb[:256].reshape(2, 128, NBLK, SBLK).transpose(2, 1, 0, 3)
        ).reshape(NBLK * 128, -1)
        xj2 = np.empty((NJ2, S), dtype=BF16)
        xj2[0:62] = xb[256:]
        xj2[62:62 + LOC] = l_[b].T * XS
        xj2[62 + LOC] = XS
        m["xhbj2"] = np.ascontiguousarray(
            xj2.reshape(NJ2, NBLK, SBLK).transpose(1, 0, 2)
        ).reshape(NBLK * NJ2, -1)
        wbj2 = wbj2_base.copy()
        # ones-row carries XS, so the bias row needs only WS.
        wbj2[62 + LOC, 0] = bq_eff[b] * WS
        wbj2[62 + LOC, 1] = bk_eff[b] * WS
        m["wbj2"] = wbj2.astype(BF16).reshape(NJ2, -1)
        in_maps.append(m)
    return in_maps


def kernel(h, mask, g, l, Wq, bq, Wk, bk, Wv=None, bv=None):
    from concourse.bass_utils import run_bass_kernel_spmd

    mask = np.asarray(mask)
    in_maps = prep_in_maps(h, mask, g, l, Wq, bq, Wk, bk)

    nc = _get_nc()
    res = run_bass_kernel_spmd(nc, in_maps, core_ids=list(range(B)), trace=False)

    scores = np.empty((B, S), dtype=np.float32)
    for b in range(B):
        scores[b] = res.results[b]["out"].T.reshape(S)
    return np.where(mask == 1, np.float32(-1e9), scores).astype(np.float32)
